# revision 1
# baseline (speedup 1.0000x reference)
"""Trainium2 Bass kernel for the gnn_message_passing encoder problem.

kernel(**inputs) takes the FULL inputs and returns the FULL [B, P, R+1] output.

Sharding: 8 cores = 2 batches x 4 object-groups.  Each core scores ~62
(trigger, object) pairs of one document.  The host shards inputs per core:
only the attention rows touched by the core's entity spans are shipped
(laid out dense as [head, entity, w, L]), plus the full sequence_output of
its batch (needed by the context matmul), the span token rows, small
one-hot selection matrices (with the 1/W span-mean folded in), and the
transposed relation/nota codebooks.  All arithmetic (span means, pair
products, head sums, normalization, context matmul, scoring, NOTA max)
runs on device.
"""

import os
import sys

import numpy as np

for _p in ("/opt/trn_rl_repo", os.path.expanduser("~/.axon_site/_ro/trn_rl_repo")):
    if os.path.isdir(_p) and _p not in sys.path:
        sys.path.insert(0, _p)

import concourse.bass as bass
import concourse.mybir as mybir
import concourse.tile as tile
from concourse import bacc
from concourse.bass_utils import run_bass_kernel_spmd

# Problem dimensions (hardcoded per the harness contract).
B, L, D, H = 2, 2048, 768, 12
E, T, W = 32, 8, 4
R, NN = 57, 20
RN = R + NN            # 77 stacked codebook rows
F = 3 * D              # 2304 concat feature dim
NE = 16                # entities per core (8 triggers + 8 objects)
NEW = NE * W           # 64 gathered rows per head
NP = 64                # pair slots per core (group 0 pads 56 -> 64)
LQ = 512               # L is processed in 4 slices of 512
NCORES = 8

# Static pair list in the reference's order (s-major).
ALL_PAIRS = [(s, o) for s in range(T) for o in range(E) if s != o]
GROUP_IDX = [[i for i, (_, o) in enumerate(ALL_PAIRS) if o // 8 == g] for g in range(4)]
GROUP_ENTS = [
    list(range(16)),
    list(range(16)),
    list(range(8)) + list(range(16, 24)),
    list(range(8)) + list(range(24, 32)),
]

F32 = mybir.dt.float32
BF16 = mybir.dt.bfloat16
import ml_dtypes
NP_BF16 = ml_dtypes.bfloat16

LAST_RESULTS = None  # BassKernelResults of the most recent kernel() call

FOLD2 = np.ascontiguousarray(
    np.concatenate([np.eye(NP), np.eye(NP)], axis=0).astype(np.float32)
)


def _sel_matrices(g):
    """[NEW, NP] one-hot (x 0.25) selectors for the s and o side of each pair."""
    idxs = GROUP_IDX[g]
    ents = GROUP_ENTS[g]
    local = {e: i for i, e in enumerate(ents)}
    sel_s = np.zeros((NEW, NP), np.float32)
    sel_o = np.zeros((NEW, NP), np.float32)
    for j in range(NP):
        s, o = ALL_PAIRS[idxs[j % len(idxs)]]  # pad group 0 by repeating pair 0
        for w in range(W):
            sel_s[local[s] * W + w, j] = 0.25
            sel_o[local[o] * W + w, j] = 0.25
    return sel_s, sel_o


def _build_program():
    nc = bacc.Bacc("TRN2")

    att_g = nc.dram_tensor("att_g", [4, 128, 6 * LQ], BF16, kind="ExternalInput")
    seq = nc.dram_tensor("seq", [128, 16 * D], BF16, kind="ExternalInput")
    spans = nc.dram_tensor("spans", [NEW, D], BF16, kind="ExternalInput")
    selb = nc.dram_tensor("selb", [2 * NEW, 2 * NP], BF16, kind="ExternalInput")
    rel_t = nc.dram_tensor("rel_t", [128, 18 * RN], BF16, kind="ExternalInput")
    out = nc.dram_tensor("out", [NP, R + 1], F32, kind="ExternalOutput")

    with tile.TileContext(nc) as tc:
        with tc.tile_pool(name="consts", bufs=1) as consts:
            # Small inputs first so the early stages can start immediately.
            selb_sb = consts.tile([2 * NEW, 2, NP], BF16)
            nc.sync.dma_start(out=selb_sb, in_=selb.rearrange("p (s n) -> p s n", s=2))
            # Attention rows split by L-quarter (selb above is tiny and first) so stage A pipelines with DMA.
            g_sb = consts.tile([128, 6, L], BF16)      # [h,e,w] rows: 2 heads/tile
            g_view = att_g.rearrange("q p (t l) -> q p t l", t=6)
            nc.sync.dma_start(out=g_sb[:, :, 0:LQ], in_=g_view[0])
            spans_sb = consts.tile([NEW, D], BF16)
            nc.sync.dma_start(out=spans_sb, in_=spans[:, :])
            nc.sync.dma_start(out=g_sb[:, :, LQ:2 * LQ], in_=g_view[1])
            nc.sync.dma_start(out=g_sb[:, :, 2 * LQ:3 * LQ], in_=g_view[2])
            nc.sync.dma_start(out=g_sb[:, :, 3 * LQ:4 * LQ], in_=g_view[3])
            seq_sb = consts.tile([128, 16, D], BF16)   # 16 L-chunks of [128, D]
            nc.sync.dma_start(out=seq_sb, in_=seq.rearrange("p (c d) -> p c d", c=16))
            rel_sb = consts.tile([128, 18, RN], BF16)  # 18 K-chunks of [128, RN]
            nc.sync.dma_start(out=rel_sb, in_=rel_t.rearrange("p (c n) -> p c n", c=18))
            # fold2 = [I64; I64] built on device
            fold2_sb = consts.tile([128, NP], BF16)
            nc.gpsimd.memset(fold2_sb, 0.0)
            nc.gpsimd.affine_select(
                out=fold2_sb[0:NP, :], in_=fold2_sb[0:NP, :],
                compare_op=mybir.AluOpType.not_equal, fill=1.0, base=0,
                pattern=[[-1, NP]], channel_multiplier=1,
            )
            nc.gpsimd.affine_select(
                out=fold2_sb[NP:128, :], in_=fold2_sb[NP:128, :],
                compare_op=mybir.AluOpType.not_equal, fill=1.0, base=0,
                pattern=[[-1, NP]], channel_multiplier=1,
            )
            id_sb = consts.tile([128, 128], F32)
            nc.gpsimd.memset(id_sb, 0.0)
            nc.gpsimd.affine_select(
                out=id_sb, in_=id_sb,
                compare_op=mybir.AluOpType.not_equal, fill=1.0, base=0,
                pattern=[[-1, 128]], channel_multiplier=1,
            )

            q_sb = consts.tile([NP, L], F32)
            aT_sb = consts.tile([128, 16, NP], BF16)
            embsT = consts.tile([128, 18, NP], BF16)
            fin = consts.tile([NP, R + 1], F32)

            # PSUM budget: psA(2x2=4) + psQ(2) + psC(2) = 8 banks.
            with tc.tile_pool(name="psA", bufs=2, space="PSUM") as psA, \
                 tc.tile_pool(name="psQ", bufs=2, space="PSUM") as psQ, \
                 tc.tile_pool(name="psC", bufs=1, space="PSUM") as psC, \
                 tc.tile_pool(name="prod", bufs=3) as prod:
                c_ps0 = psC.tile([NP, 384], F32, tag="c0")
                c_ps1 = psC.tile([NP, 384], F32, tag="c1")


                # Stage A: per-pair attention rows via one-hot matmuls
                # (2 heads stacked per PSUM tile), pair products + head sums.
                qp4 = consts.tile([NP, 4], F32)
                for lq in range(4):
                    pm = prod.tile([128, 6, LQ], BF16, tag="prods")
                    for hp in range(6):
                        a_s = psA.tile([128, LQ], F32, tag="as")
                        a_o = psA.tile([128, LQ], F32, tag="ao")
                        lo = g_sb[0:64, hp, lq * LQ:(lq + 1) * LQ]
                        hi = g_sb[64:128, hp, lq * LQ:(lq + 1) * LQ]
                        nc.tensor.matmul(out=a_s[0:64, :], lhsT=selb_sb[0:64, 0, :], rhs=lo)
                        nc.tensor.matmul(out=a_s[64:128, :], lhsT=selb_sb[64:128, 0, :], rhs=hi)
                        nc.tensor.matmul(out=a_o[0:64, :], lhsT=selb_sb[0:64, 1, :], rhs=lo)
                        nc.tensor.matmul(out=a_o[64:128, :], lhsT=selb_sb[64:128, 1, :], rhs=hi)
                        # 2-input DVE ops may read at most one PSUM operand:
                        # stage the S side through SBUF on the scalar engine
                        as_sb = prod.tile([128, LQ], F32, tag="as_sb")
                        nc.scalar.copy(as_sb, a_s)
                        nc.vector.tensor_mul(pm[:, hp, :], as_sb, a_o)
                    # head-sum tree, merged ops
                    nc.vector.tensor_add(pm[:, 0:3, :], pm[:, 0:3, :], pm[:, 3:6, :])
                    nc.vector.tensor_add(pm[:, 0, :], pm[:, 0, :], pm[:, 1, :])
                    nc.vector.tensor_add(pm[:, 0, :], pm[:, 0, :], pm[:, 2, :])
                    # fold the two stacked head-halves across partitions
                    qt = psQ.tile([NP, LQ], F32, tag="qe")
                    nc.tensor.matmul(out=qt, lhsT=fold2_sb, rhs=pm[:, 0, :])
                    nc.vector.reduce_sum(qp4[:, lq:lq + 1], qt, axis=mybir.AxisListType.X)
                    nc.scalar.copy(q_sb[:, lq * LQ:(lq + 1) * LQ], qt)
                    for k in range(4):
                        ch = lq * 4 + k
                        qT_ps = psQ.tile([128, NP], F32, tag="qe")
                        nc.tensor.transpose(
                            qT_ps, q_sb[:, ch * 128:(ch + 1) * 128], id_sb[0:NP, 0:NP]
                        )
                        nc.scalar.copy(aT_sb[:, ch, :], qT_ps)
                        nc.tensor.matmul(out=c_ps0, lhsT=aT_sb[:, ch, :],
                                         rhs=seq_sb[:, ch, 0:384],
                                         start=(ch == 0), stop=(ch == 15))
                        nc.tensor.matmul(out=c_ps1, lhsT=aT_sb[:, ch, :],
                                         rhs=seq_sb[:, ch, 384:768],
                                         start=(ch == 0), stop=(ch == 15))


                # Stage E: entity embeddings (span mean folded into selectors).
                # Emitted after stage A; the scheduler slots these into gaps.
                for dt in range(6):
                    ee_ps = psQ.tile([128, 2 * NP], F32, tag="qe")
                    sp_sl = spans_sb[:, dt * 128:(dt + 1) * 128]
                    nc.tensor.matmul(out=ee_ps[:, 0:NP], lhsT=sp_sl, rhs=selb_sb[0:64, 0, :])
                    nc.tensor.matmul(out=ee_ps[:, NP:2 * NP], lhsT=sp_sl, rhs=selb_sb[0:64, 1, :])
                    nc.scalar.copy(embsT[:, dt, :], ee_ps[:, 0:NP])
                    nc.scalar.copy(embsT[:, 6 + dt, :], ee_ps[:, NP:2 * NP])
                # Stage B: 1/rowsum(q)
                qsum = consts.tile([NP, 1], F32)
                nc.vector.reduce_sum(qsum, qp4, axis=mybir.AxisListType.X)
                rq = consts.tile([NP, 1], F32)
                nc.vector.reciprocal(rq, qsum)

            with tc.tile_pool(name="psF", bufs=1, space="PSUM") as psF:
                # normalize by 1/rowsum(q) (per-partition scalar), transpose
                # into embsT layout [d, p] (bf16)
                c_sb = consts.tile([NP, D], F32)
                nc.vector.tensor_scalar_mul(c_sb[:, 0:384], c_ps0, rq)
                nc.vector.tensor_scalar_mul(c_sb[:, 384:768], c_ps1, rq)
                for dt in range(6):
                    cT_ps = psF.tile([128, NP], F32, tag="cT", bufs=2)
                    nc.tensor.transpose(
                        cT_ps, c_sb[:, dt * 128:(dt + 1) * 128], id_sb[0:NP, 0:NP]
                    )
                    nc.vector.tensor_copy(embsT[:, 12 + dt, :], cT_ps)
                # Stage F: scores = [rel; nota] @ embs, then transpose + NOTA max
                sc_ps = psF.tile([RN, NP], F32, tag="sc")
                for kc in range(18):
                    nc.tensor.matmul(
                        out=sc_ps,
                        lhsT=rel_sb[:, kc, :],
                        rhs=embsT[:, kc, :],
                        start=(kc == 0),
                        stop=(kc == 17),
                    )
                sc_sb = consts.tile([RN, NP], F32)
                nc.vector.tensor_copy(sc_sb, sc_ps)
                scT_ps = psF.tile([NP, RN], F32, tag="scT")
                nc.tensor.transpose(scT_ps, sc_sb, id_sb[0:RN, 0:RN])
                nota = consts.tile([NP, 1], F32)
                nc.vector.reduce_max(nota, scT_ps[:, R:RN], axis=mybir.AxisListType.X)
                nc.vector.tensor_copy(fin[:, 1:R + 1], scT_ps[:, 0:R])
                nc.vector.tensor_copy(fin[:, 0:1], nota)

            nc.sync.dma_start(out=out[:, :], in_=fin)

    return nc


def kernel(sequence_output, attention, relation_embeddings, nota_embeddings,
           span_starts):
    global LAST_RESULTS
    sequence_output = np.asarray(sequence_output, np.float32)
    attention = np.asarray(attention, np.float32)
    span_starts = np.asarray(span_starts)
    rel_t = np.ascontiguousarray(
        np.concatenate(
            [np.asarray(relation_embeddings, np.float32),
             np.asarray(nota_embeddings, np.float32)], axis=0
        ).T
    )

    in_maps = []
    for c in range(NCORES):
        b, g = divmod(c, 4)
        ents = GROUP_ENTS[g]
        rows = np.concatenate(
            [np.arange(span_starts[b, e], span_starts[b, e] + W) for e in ents]
        )
        att_rows = attention[b][:, rows, :].reshape(H * NEW, L)
        sel_s, sel_o = _sel_matrices(g)
        att_q = att_rows.astype(NP_BF16).reshape(6, 128, 4, LQ).transpose(2, 1, 0, 3)
        seq_pm = sequence_output[b].astype(NP_BF16).reshape(16, 128, D).transpose(1, 0, 2).reshape(128, 16 * D)
        in_maps.append({
            "att_g": np.ascontiguousarray(att_q.reshape(4, 128, 6 * LQ)),
            "seq": np.ascontiguousarray(seq_pm),
            "spans": np.ascontiguousarray(sequence_output[b][rows].astype(NP_BF16)),
            "selb": np.ascontiguousarray(
                np.stack([np.concatenate([sel_s, sel_s], 0),
                          np.concatenate([sel_o, sel_o], 0)], axis=1
                         ).reshape(2 * NEW, 2 * NP).astype(NP_BF16)),
            "rel_t": np.ascontiguousarray(rel_t.astype(NP_BF16).reshape(18, 128, RN).transpose(1, 0, 2).reshape(128, 18 * RN)),
        })

    nc = _build_program()
    nc.finalize()  # Bacc legalization (wait splitting, reg alloc)
    LAST_RESULTS = run_bass_kernel_spmd(nc, in_maps, core_ids=list(range(NCORES)))

    out = np.zeros((B, len(ALL_PAIRS), R + 1), np.float32)
    for c in range(NCORES):
        b, g = divmod(c, 4)
        idxs = GROUP_IDX[g]
        out[b, idxs, :] = LAST_RESULTS.results[c]["out"][: len(idxs)]
    return out



# revision 8
# speedup vs baseline: 1.3434x; 1.3434x over previous
"""Trainium2 Bass kernel for the gnn_message_passing encoder problem.

kernel(**inputs) takes the FULL inputs and returns the FULL [B, P, R+1] output.

Sharding: 8 cores = 2 batches x 4 object-groups; each core scores 64
(trigger, object) pair slots of one document (group 0 pads its 56 valid
pairs to 64).  The host does layout only (gather of the needed attention
rows, transposes, dtype casts); all arithmetic runs on device.

Device-side layout: sequence positions l ride the SBUF partition dim
(16 tiles of 128).  Per tile the attention rows arrive as [l, (w4 e16 h12)]
so that the span-width sum is two strided adds, the per-pair product
A[s,o,h,l] = Xs[s,h,l]*Xo[o,h,l] is ONE broadcast tensor_tensor mul
(heads innermost keeps every operand packed => DVE 2x mode), and the
head-sum is a small add tree.  The resulting A[l, pair] is directly the
lhsT of the context matmul against seq[l, d] -- no transposes at all on
the main path.  q rides the matmul as an appended ones-column of seq.
Scoring contracts pair embeddings against pre-transposed codebook chunks.
"""

import os
import sys

import numpy as np

for _p in ("/opt/trn_rl_repo", os.path.expanduser("~/.axon_site/_ro/trn_rl_repo")):
    if os.path.isdir(_p) and _p not in sys.path:
        sys.path.insert(0, _p)

import concourse.bass as bass
import concourse.mybir as mybir
import concourse.tile as tile
from concourse import bacc
from concourse.bass_utils import run_bass_kernel_spmd

# Problem dimensions (hardcoded per the harness contract).
B, L, D, H = 2, 2048, 768, 12
E, T, W = 32, 8, 4
R, NN = 57, 20
RN = R + NN            # 77 stacked codebook rows
NE = 16                # entities per core: 8 triggers + 8 objects
NP = 64                # pair slots per core (8 s x 8 o)
NT = 16                # L tiles of 128
NQ = 4                 # quads of 4 tiles
NCORES = 8

# Reference pair order: s-major, o-minor, skip s==o.
ALL_PAIRS = [(s, o) for s in range(T) for o in range(E) if s != o]
GROUP_IDX = [[i for i, (_, o) in enumerate(ALL_PAIRS) if o // 8 == g] for g in range(4)]

F32 = mybir.dt.float32
BF16 = mybir.dt.bfloat16
import ml_dtypes
NP_BF16 = ml_dtypes.bfloat16

LAST_RESULTS = None  # BassKernelResults of the most recent kernel() call


def _build_program():
    nc = bacc.Bacc("TRN2")

    # DRAM inputs (per-core shards, host-prepared).
    attq = nc.dram_tensor("attq", [128, NT * W * NE * H], BF16, kind="ExternalInput")
    seqq = nc.dram_tensor("seqq", [128, NT * (D + 1)], BF16, kind="ExternalInput")
    spansT = nc.dram_tensor("spansT", [128, 6 * NP], BF16, kind="ExternalInput")
    relq = nc.dram_tensor("relq", [128, 18 * RN], BF16, kind="ExternalInput")
    out = nc.dram_tensor("out", [NP, R + 1], F32, kind="ExternalOutput")

    QW = 4 * W * NE * H      # free elems per att quad   (3072)
    QS = 4 * (D + 1)         # free elems per seq quad   (3076)

    with tile.TileContext(nc) as tc:
        with tc.tile_pool(name="consts", bufs=1) as consts, \
             tc.tile_pool(name="attp", bufs=2) as attp, \
             tc.tile_pool(name="seqp", bufs=2) as seqp, \
             tc.tile_pool(name="work", bufs=2) as work:

            # Small tensors first so the scoring side-path can start early.
            spansT_sb = consts.tile([128, 6, NP], BF16)
            nc.sync.dma_start(out=spansT_sb, in_=spansT.rearrange("p (k n) -> p k n", k=6))
            relq_sb = consts.tile([128, 18, RN], BF16)
            nc.sync.dma_start(out=relq_sb, in_=relq.rearrange("p (k n) -> p k n", k=18))

            A_sb = consts.tile([128, NT, NP], BF16)     # head-summed pair rows (c lhsT)
            id_bf = consts.tile([128, 128], BF16)
            nc.gpsimd.memset(id_bf, 0.0)
            nc.gpsimd.affine_select(
                out=id_bf, in_=id_bf,
                compare_op=mybir.AluOpType.not_equal, fill=1.0, base=0,
                pattern=[[-1, 128]], channel_multiplier=1,
            )
            id_f32 = consts.tile([RN, RN], F32)
            nc.gpsimd.memset(id_f32, 0.0)
            nc.gpsimd.affine_select(
                out=id_f32, in_=id_f32,
                compare_op=mybir.AluOpType.not_equal, fill=1.0, base=0,
                pattern=[[-1, RN]], channel_multiplier=1,
            )

            with tc.tile_pool(name="psC", bufs=1, space="PSUM") as psC, \
                 tc.tile_pool(name="psS", bufs=1, space="PSUM") as psS, \
                 tc.tile_pool(name="psT", bufs=2, space="PSUM") as psT:
                c_ps0 = psC.tile([NP, 384], F32, tag="c0")
                c_ps1 = psC.tile([NP, 385], F32, tag="c1")

                # --- entity scoring side-path (tiny, PE + a couple DVE folds)
                # rS/rO: per-span-row scores vs rel segment d 0:768 / 768:1536.
                sc_e = psS.tile([RN, 2 * NP], F32, tag="sce")
                for k in range(6):
                    nc.tensor.matmul(out=sc_e[:, 0:NP], lhsT=relq_sb[:, k, :],
                                     rhs=spansT_sb[:, k, :], start=(k == 0), stop=(k == 5))
                for k in range(6):
                    nc.tensor.matmul(out=sc_e[:, NP:2 * NP], lhsT=relq_sb[:, 6 + k, :],
                                     rhs=spansT_sb[:, k, :], start=(k == 0), stop=(k == 5))
                # fold span width 4 -> 1 (mean/4 deferred: fold=sum, scale later)
                sc_e_sb = consts.tile([RN, 2 * NP], F32)
                nc.scalar.copy(sc_e_sb, sc_e)
                eSO2 = consts.tile([RN, 2, NE, 2], F32)
                eSO = consts.tile([RN, 2, NE], F32)
                v = sc_e_sb.rearrange("r (x e w) -> r x e w", x=2, w=W)
                nc.vector.tensor_add(eSO2, v[:, :, :, 0:2], v[:, :, :, 2:4])
                nc.vector.tensor_add(eSO, eSO2[:, :, :, 0], eSO2[:, :, :, 1])

                # --- main path: per quad of 4 L-tiles
                for q in range(NQ):
                    att_sb = attp.tile([128, 4, W, NE, H], BF16, tag="att")
                    nc.sync.dma_start(
                        out=att_sb,
                        in_=attq.rearrange("p (t w e h) -> p t w e h",
                                           t=NT, w=W, e=NE)[:, 4 * q:4 * q + 4],
                    )
                    seq_sb = seqp.tile([128, 4, D + 1], BF16, tag="seq")
                    nc.sync.dma_start(
                        out=seq_sb,
                        in_=seqq.rearrange("p (t d) -> p t d", t=NT)[:, 4 * q:4 * q + 4],
                    )

                    # span-width sum: (w0+w2)+(w1+w3)  [all packed, 2x]
                    wf = work.tile([128, 4, 2, NE * H], BF16, tag="wf")
                    nc.vector.tensor_add(
                        wf,
                        att_sb.rearrange("p t w e h -> p t w (e h)")[:, :, 0:2],
                        att_sb.rearrange("p t w e h -> p t w (e h)")[:, :, 2:4],
                    )
                    X = work.tile([128, 4, NE, H], BF16, tag="X")
                    nc.vector.tensor_add(
                        X.rearrange("p t e h -> p t (e h)"),
                        wf[:, :, 0], wf[:, :, 1],
                    )
                    # pair products: [l, t, s, o, h] = Xs * Xo (broadcast APs;
                    # TENSOR3D allows only 3 free dims -> one mul per tile)
                    prod = work.tile([128, 4, T, 8, H], BF16, tag="prod")
                    for i in range(4):
                        nc.vector.tensor_mul(
                            prod[:, i],
                            X[:, i, 0:T, :].unsqueeze(2).broadcast_to([128, T, 8, H]),
                            X[:, i, T:NE, :].unsqueeze(1).broadcast_to([128, T, 8, H]),
                        )
                    # head-sum tree 12 -> 6 -> 3 -> 1
                    h6 = work.tile([128, 4, NP, 6], BF16, tag="h6")
                    pv = prod.rearrange("p t s o h -> p t (s o) h")
                    nc.vector.tensor_add(h6, pv[:, :, :, 0:6], pv[:, :, :, 6:12])
                    h3 = work.tile([128, 4, NP, 3], BF16, tag="h3")
                    nc.vector.tensor_add(h3, h6[:, :, :, 0:3], h6[:, :, :, 3:6])
                    Aq = A_sb[:, 4 * q:4 * q + 4, :]
                    nc.gpsimd.tensor_add(Aq, h3[:, :, :, 0], h3[:, :, :, 1])
                    nc.gpsimd.tensor_add(Aq, Aq, h3[:, :, :, 2])

                    # context matmul accumulation for this quad's 4 tiles
                    for i in range(4):
                        t = 4 * q + i
                        nc.tensor.matmul(out=c_ps0, lhsT=A_sb[:, t, :],
                                         rhs=seq_sb[:, i, 0:384],
                                         start=(t == 0), stop=(t == NT - 1))
                        nc.tensor.matmul(out=c_ps1, lhsT=A_sb[:, t, :],
                                         rhs=seq_sb[:, i, 384:769],
                                         start=(t == 0), stop=(t == NT - 1))

                # --- normalize context by 1/q  (q = ones-column, col 384 of c_ps1)
                rq = consts.tile([NP, 1], F32)
                nc.vector.reciprocal(rq, c_ps1[:, 384:385])
                c_norm = consts.tile([NP, D], BF16)
                nc.vector.tensor_scalar_mul(c_norm[:, 0:384], c_ps0, rq)
                nc.vector.tensor_scalar_mul(c_norm[:, 384:768], c_ps1[:, 0:384], rq)

                # transpose c_norm into [d, p] chunks for scoring
                cT_sb = consts.tile([128, 6, NP], BF16)
                for k in range(6):
                    cT_ps = psT.tile([128, NP], BF16, tag="cT")
                    nc.tensor.transpose(cT_ps, c_norm[:, 128 * k:128 * (k + 1)],
                                        id_bf[0:NP, 0:NP])
                    nc.vector.tensor_copy(cT_sb[:, k, :], cT_ps)

                # c scores + final assembly in [rn, p] layout
                sc_c = psS.tile([RN, NP], F32, tag="scc")
                for k in range(6):
                    nc.tensor.matmul(out=sc_c, lhsT=relq_sb[:, 12 + k, :],
                                     rhs=cT_sb[:, k, :], start=(k == 0), stop=(k == 5))
                fin_T = consts.tile([RN, NP], F32)
                # + 0.25*eS[s(p)] + 0.25*eO[o(p)]  (gathers via broadcast APs)
                nc.vector.scalar_tensor_tensor(
                    out=fin_T.rearrange("r (s o) -> r s o", s=T),
                    in0=eSO[:, 0, 0:T].unsqueeze(2).broadcast_to([RN, T, 8]),
                    scalar=0.25, in1=sc_c.rearrange("r (s o) -> r s o", s=T),
                    op0=mybir.AluOpType.mult, op1=mybir.AluOpType.add,
                )
                fin_T2 = consts.tile([RN, NP], F32)
                nc.vector.scalar_tensor_tensor(
                    out=fin_T2.rearrange("r (s o) -> r s o", s=T),
                    in0=eSO[:, 1, T:NE].unsqueeze(1).broadcast_to([RN, T, 8]),
                    scalar=0.25, in1=fin_T.rearrange("r (s o) -> r s o", s=T),
                    op0=mybir.AluOpType.mult, op1=mybir.AluOpType.add,
                )

                # transpose to [p, rn], NOTA max, assemble [p, 1+R]
                finT_ps = psT.tile([NP, RN], F32, tag="fT")
                nc.tensor.transpose(finT_ps, fin_T2, id_f32)
                res = consts.tile([NP, R + 1], F32)
                nc.vector.reduce_max(res[:, 0:1], finT_ps[:, R:RN],
                                     axis=mybir.AxisListType.X)
                nc.vector.tensor_copy(res[:, 1:R + 1], finT_ps[:, 0:R])
                nc.sync.dma_start(out=out[:, :], in_=res)

    return nc


def _host_shards(sequence_output, attention, relation_embeddings, nota_embeddings,
                 span_starts):
    rel_all = np.concatenate(
        [np.asarray(relation_embeddings, np.float32),
         np.asarray(nota_embeddings, np.float32)], axis=0)          # [77, 2304]
    relq = np.ascontiguousarray(
        rel_all.T.reshape(18, 128, RN).transpose(1, 0, 2).reshape(128, 18 * RN)
    ).astype(NP_BF16)

    in_maps = []
    for c in range(NCORES):
        b, g = divmod(c, 4)
        obj = list(range(8)) if g == 0 else list(range(8 * g, 8 * g + 8))
        ents = list(range(T)) + obj
        rows = np.concatenate(
            [np.arange(span_starts[b, e], span_starts[b, e] + W) for e in ents]
        )
        # attention rows -> [l, w, e, h] -> [128, (t w e h)]
        att_rows = attention[b][:, rows, :]                          # [H, 64, L]
        att_t = att_rows.reshape(H, NE, W, L).transpose(3, 2, 1, 0)  # [L, w, e, h]
        attq = (att_t.reshape(NT, 128, W * NE * H).transpose(1, 0, 2)
                .reshape(128, NT * W * NE * H)).astype(NP_BF16)
        # seq tiles + ones column -> [128, (t, 769)]
        st = sequence_output[b].reshape(NT, 128, D)
        st = np.concatenate([st, np.ones((NT, 128, 1), np.float32)], axis=2)
        seqq = st.transpose(1, 0, 2).reshape(128, NT * (D + 1)).astype(NP_BF16)
        # span rows transposed -> [d, row] chunks [128, (6, 64)]
        spT = sequence_output[b][rows].T.reshape(6, 128, NP).transpose(1, 0, 2)
        spansT = spT.reshape(128, 6 * NP).astype(NP_BF16)
        in_maps.append({
            "attq": np.ascontiguousarray(attq),
            "seqq": np.ascontiguousarray(seqq),
            "spansT": np.ascontiguousarray(spansT),
            "relq": relq,
        })
    return in_maps


def kernel(sequence_output, attention, relation_embeddings, nota_embeddings,
           span_starts):
    global LAST_RESULTS
    sequence_output = np.asarray(sequence_output, np.float32)
    attention = np.asarray(attention, np.float32)
    span_starts = np.asarray(span_starts)

    in_maps = _host_shards(sequence_output, attention, relation_embeddings,
                           nota_embeddings, span_starts)

    nc = _build_program()
    nc.finalize()
    LAST_RESULTS = run_bass_kernel_spmd(nc, in_maps, core_ids=list(range(NCORES)))

    out = np.zeros((B, len(ALL_PAIRS), R + 1), np.float32)
    for c in range(NCORES):
        b, g = divmod(c, 4)
        res = LAST_RESULTS.results[c]["out"]          # [64, 78], p = s*8+o_local
        idxs = GROUP_IDX[g]
        rows = [s * 8 + (o % 8) for (s, o) in (ALL_PAIRS[i] for i in idxs)]
        out[b, idxs, :] = res[rows]
    return out


# revision 24
# speedup vs baseline: 1.4909x; 1.1098x over previous
"""Trainium2 Bass kernel for the gnn_message_passing encoder problem.

kernel(**inputs) takes the FULL inputs and returns the FULL [B, P, R+1] output.

Sharding: 8 cores = 2 batches x 4 object-groups; each core scores 64
(trigger, object) pair slots of one document (group 0 pads its 56 valid
pairs to 64).  The host does layout only (gather of the needed attention
rows, transposes, dtype casts); all arithmetic runs on device.

Device-side layout: sequence positions l ride the SBUF partition dim
(16 tiles of 128).  Per tile the attention rows arrive as [l, (w4 e16 h12)]
so that the span-width sum is two strided adds, the per-pair product
A[s,o,h,l] = Xs[s,h,l]*Xo[o,h,l] is ONE broadcast tensor_tensor mul
(heads innermost keeps every operand packed => DVE 2x mode), and the
head-sum is a small add tree.  The resulting A[l, pair] is directly the
lhsT of the context matmul against seq[l, d] -- no transposes at all on
the main path.  q rides the matmul as an appended ones-column of seq.
Scoring contracts pair embeddings against pre-transposed codebook chunks.
"""

import os
import sys

import numpy as np

for _p in ("/opt/trn_rl_repo", os.path.expanduser("~/.axon_site/_ro/trn_rl_repo")):
    if os.path.isdir(_p) and _p not in sys.path:
        sys.path.insert(0, _p)

import concourse.bass as bass
import concourse.mybir as mybir
import concourse.tile as tile
from concourse import bacc
from concourse.bass_utils import run_bass_kernel_spmd

# Problem dimensions (hardcoded per the harness contract).
B, L, D, H = 2, 2048, 768, 12
E, T, W = 32, 8, 4
R, NN = 57, 20
RN = R + NN            # 77 stacked codebook rows
NE = 16                # entities per core: 8 triggers + 8 objects
NP = 64                # pair slots per core (8 s x 8 o)
NT = 16                # L tiles of 128
NQ = 4                 # quads of 4 tiles
NCORES = 8

# Reference pair order: s-major, o-minor, skip s==o.
ALL_PAIRS = [(s, o) for s in range(T) for o in range(E) if s != o]
GROUP_IDX = [[i for i, (_, o) in enumerate(ALL_PAIRS) if o // 8 == g] for g in range(4)]

F32 = mybir.dt.float32
BF16 = mybir.dt.bfloat16
import ml_dtypes
NP_BF16 = ml_dtypes.bfloat16

LAST_RESULTS = None  # BassKernelResults of the most recent kernel() call


def _build_program():
    nc = bacc.Bacc("TRN2")

    # DRAM inputs (per-core shards, host-prepared).
    attq = nc.dram_tensor("attq", [128, NT * W * NE * H], BF16, kind="ExternalInput")
    seqq = nc.dram_tensor("seqq", [128, NT * (D + 1)], BF16, kind="ExternalInput")
    spansT = nc.dram_tensor("spansT", [128, 6 * NP], BF16, kind="ExternalInput")
    relq = nc.dram_tensor("relq", [128, 18 * RN], BF16, kind="ExternalInput")
    out = nc.dram_tensor("out", [NP, R + 1], F32, kind="ExternalOutput")

    QW = 4 * W * NE * H      # free elems per att quad   (3072)
    QS = 4 * (D + 1)         # free elems per seq quad   (3076)

    with tile.TileContext(nc) as tc:
        with tc.tile_pool(name="consts", bufs=1) as consts, \
             tc.tile_pool(name="attp", bufs=4) as attp, \
             tc.tile_pool(name="seqp", bufs=4) as seqp, \
             tc.tile_pool(name="work", bufs=2) as work:

            # Main-path DMAs first (critical path); att quads from the sync
            # sequencer, seq quads from the scalar sequencer, in parallel.
            # bufs=4 on att/seq pools => no buffer-reuse stalls, DMA engines
            # stream the full input while compute chases quad by quad.
            att_sbs, seq_sbs = [], []
            attv = attq.rearrange("p (t w e h) -> p t w e h", t=NT, w=W, e=NE)
            seqv = seqq.rearrange("p (t d) -> p t d", t=NT)
            for q in range(2):
                att_sb = attp.tile([128, 4, W, NE, H], BF16, tag="att")
                nc.sync.dma_start(out=att_sb, in_=attv[:, 4 * q:4 * q + 4])
                seq_sb = seqp.tile([128, 4, D + 1], BF16, tag="seq")
                nc.scalar.dma_start(out=seq_sb, in_=seqv[:, 4 * q:4 * q + 4])
                att_sbs.append(att_sb)
                seq_sbs.append(seq_sb)
            relq_sb = consts.tile([128, 18, RN], BF16)
            nc.sync.dma_start(out=relq_sb, in_=relq.rearrange("p (k n) -> p k n", k=18))
            spansT_sb = consts.tile([128, 6, NP], BF16)
            nc.scalar.dma_start(out=spansT_sb, in_=spansT.rearrange("p (k n) -> p k n", k=6))
            for q in range(2, NQ):
                att_sb = attp.tile([128, 4, W, NE, H], BF16, tag="att")
                nc.sync.dma_start(out=att_sb, in_=attv[:, 4 * q:4 * q + 4])
                seq_sb = seqp.tile([128, 4, D + 1], BF16, tag="seq")
                nc.scalar.dma_start(out=seq_sb, in_=seqv[:, 4 * q:4 * q + 4])
                att_sbs.append(att_sb)
                seq_sbs.append(seq_sb)

            A_sb = consts.tile([128, NT, NP, 2], BF16)  # pair rows, heads folded to 2
            id_f32 = consts.tile([RN, RN], F32)
            nc.gpsimd.memset(id_f32, 0.0)
            nc.gpsimd.affine_select(
                out=id_f32, in_=id_f32,
                compare_op=mybir.AluOpType.not_equal, fill=1.0, base=0,
                pattern=[[-1, RN]], channel_multiplier=1,
            )
            # foldI[m, n] = 1 iff m//2 == n: transposing-matmul against it
            # folds the interleaved (pair, h2) context PSUM rows.
            foldI = consts.tile([128, NP], BF16)
            nc.gpsimd.memset(foldI, 0.0)
            nc.gpsimd.affine_select(
                out=foldI, in_=foldI,
                compare_op=mybir.AluOpType.not_equal, fill=1.0, base=0,
                pattern=[[-2, NP]], channel_multiplier=1,
            )
            nc.gpsimd.affine_select(
                out=foldI, in_=foldI,
                compare_op=mybir.AluOpType.not_equal, fill=1.0, base=-1,
                pattern=[[-2, NP]], channel_multiplier=1,
            )

            with tc.tile_pool(name="psC", bufs=1, space="PSUM") as psC, \
                 tc.tile_pool(name="psS", bufs=1, space="PSUM") as psS, \
                 tc.tile_pool(name="psT", bufs=1, space="PSUM") as psT:
                c_ps0 = psC.tile([128, 384], F32, tag="c0")
                c_ps1 = psC.tile([128, 385], F32, tag="c1")

                # --- entity scoring side-path (tiny, PE + a couple DVE folds)
                # rS/rO: per-span-row scores vs rel segment d 0:768 / 768:1536.
                sc_e = psS.tile([RN, 2 * NP], F32, tag="sce")
                for k in range(6):
                    nc.tensor.matmul(out=sc_e[:, 0:NP], lhsT=relq_sb[:, k, :],
                                     rhs=spansT_sb[:, k, :], start=(k == 0), stop=(k == 5))
                for k in range(6):
                    nc.tensor.matmul(out=sc_e[:, NP:2 * NP], lhsT=relq_sb[:, 6 + k, :],
                                     rhs=spansT_sb[:, k, :], start=(k == 0), stop=(k == 5))
                # fold span width 4 -> 1 (mean/4 deferred: fold=sum, scale later)
                sc_e_sb = consts.tile([RN, 2 * NP], F32)
                nc.scalar.copy(sc_e_sb, sc_e)
                eSO2 = consts.tile([RN, 2, NE, 2], F32)
                eSO = consts.tile([RN, 2, NE], F32)
                v = sc_e_sb.rearrange("r (x e w) -> r x e w", x=2, w=W)
                nc.vector.tensor_add(eSO2, v[:, :, :, 0:2], v[:, :, :, 2:4])
                nc.vector.tensor_add(eSO, eSO2[:, :, :, 0], eSO2[:, :, :, 1])

                # --- main path: per quad of 4 L-tiles
                for q in range(NQ):
                    att_sb = att_sbs[q]
                    seq_sb = seq_sbs[q]

                    # span-width sum: (w01)+(w23)  [all packed, 2x]
                    wf = work.tile([128, 4, 2, NE * H], BF16, tag="wf")
                    nc.vector.tensor_add(
                        wf,
                        att_sb.rearrange("p t w e h -> p t w (e h)")[:, :, 0:2],
                        att_sb.rearrange("p t w e h -> p t w (e h)")[:, :, 2:4],
                    )
                    X = work.tile([128, 4, NE, H], BF16, tag="X")
                    nc.vector.tensor_add(
                        X.rearrange("p t e h -> p t (e h)"),
                        wf[:, :, 0], wf[:, :, 1],
                    )
                    # pair products: [l, t, s, o, h] = Xs * Xo (broadcast APs;
                    # TENSOR3D allows only 3 free dims -> one mul per tile)
                    prod = work.tile([128, 4, T, 8, H], BF16, tag="prod")
                    for i in range(4):
                        nc.vector.tensor_mul(
                            prod[:, i],
                            X[:, i, 0:T, :].unsqueeze(2).broadcast_to([128, T, 8, H]),
                            X[:, i, T:NE, :].unsqueeze(1).broadcast_to([128, T, 8, H]),
                        )
                    # head-sum tree 12 -> 6 -> 2 (the last 2-fold rides the
                    # context matmul's M dim and is folded after PSUM)
                    h6 = work.tile([128, 4, NP, 6], BF16, tag="h6")
                    pv = prod.rearrange("p t s o h -> p t (s o) h")
                    nc.vector.tensor_add(h6, pv[:, :, :, 0:6], pv[:, :, :, 6:12])
                    Aq = A_sb[:, 4 * q:4 * q + 4, :, :]
                    nc.vector.tensor_add(Aq, h6[:, :, :, 0:2], h6[:, :, :, 2:4])
                    nc.vector.tensor_add(Aq, Aq, h6[:, :, :, 4:6])

                    # context matmul accumulation for this quad's 4 tiles
                    # (M = 128 = (pair, h2) interleaved; h2 halves fold after the
                    # transpose, where pairs sit in the free dim)
                    for i in range(4):
                        t = 4 * q + i
                        lhs = A_sb[:, t].rearrange("p n h -> p (n h)")  # [128, (pair, h2)]
                        nc.tensor.matmul(out=c_ps0, lhsT=lhs,
                                         rhs=seq_sb[:, i, 0:384],
                                         start=(t == 0), stop=(t == NT - 1))
                        nc.tensor.matmul(out=c_ps1, lhsT=lhs,
                                         rhs=seq_sb[:, i, 384:769],
                                         start=(t == 0), stop=(t == NT - 1))

                # --- PSUM -> SBUF (cast bf16), then transpose-and-fold via
                # regular matmuls against fold2 ([d, (h2, pair)]^T @ [I;I]).
                c2_sb = consts.tile([128, D + 1], BF16)
                nc.scalar.copy(c2_sb[:, 0:384], c_ps0)
                nc.scalar.copy(c2_sb[:, 384:769], c_ps1)
                cT_ps = psT.tile([128, 7, NP], F32, tag="cT")
                for k in range(6):
                    nc.tensor.matmul(out=cT_ps[:, k, :],
                                     lhsT=c2_sb[:, 128 * k:128 * (k + 1)],
                                     rhs=foldI)
                # q column (768) gets the same transpose+fold (M=1)
                nc.tensor.matmul(out=cT_ps[0:1, 6, :], lhsT=c2_sb[:, 768:769],
                                 rhs=foldI)
                cT_f = consts.tile([128, 6, NP], BF16)   # h2-folded [d, pair]
                nc.scalar.copy(cT_f, cT_ps[:, 0:6, :])
                rq_row = consts.tile([1, NP], F32)
                nc.vector.reciprocal(rq_row, cT_ps[0:1, 6, :])
                # replicate rq across RN partitions: ones[1,RN]^T (x) rq_row
                rq_bf = consts.tile([1, NP], BF16)
                nc.vector.tensor_copy(rq_bf, rq_row)
                ones_r = consts.tile([1, RN], BF16)
                nc.gpsimd.memset(ones_r, 1.0)
                rq_ps = psT.tile([RN, NP], F32, tag="rqr")
                nc.tensor.matmul(out=rq_ps, lhsT=ones_r, rhs=rq_bf)
                rq_rep = consts.tile([RN, NP], BF16)
                nc.vector.tensor_copy(rq_rep, rq_ps)

                # c scores (unnormalized) + final assembly in [rn, p] layout
                sc_c = psS.tile([RN, NP], F32, tag="scc")
                for k in range(6):
                    nc.tensor.matmul(out=sc_c, lhsT=relq_sb[:, 12 + k, :],
                                     rhs=cT_f[:, k, :], start=(k == 0), stop=(k == 5))
                fin_T = consts.tile([RN, NP], F32)
                # (c_scores * rq) + 0.25*eS[s(p)], then + 0.25*eO[o(p)]
                nc.vector.tensor_mul(fin_T, sc_c, rq_rep)
                fin_T1 = consts.tile([RN, NP], F32)
                nc.vector.scalar_tensor_tensor(
                    out=fin_T1.rearrange("r (s o) -> r s o", s=T),
                    in0=eSO[:, 0, 0:T].unsqueeze(2).broadcast_to([RN, T, 8]),
                    scalar=0.25, in1=fin_T.rearrange("r (s o) -> r s o", s=T),
                    op0=mybir.AluOpType.mult, op1=mybir.AluOpType.add,
                )
                fin_T2 = consts.tile([RN, NP], F32)
                nc.vector.scalar_tensor_tensor(
                    out=fin_T2.rearrange("r (s o) -> r s o", s=T),
                    in0=eSO[:, 1, T:NE].unsqueeze(1).broadcast_to([RN, T, 8]),
                    scalar=0.25, in1=fin_T1.rearrange("r (s o) -> r s o", s=T),
                    op0=mybir.AluOpType.mult, op1=mybir.AluOpType.add,
                )

                # transpose to [p, rn], NOTA max, assemble [p, 1+R]
                finT_ps = psT.tile([NP, RN], F32, tag="fT")
                nc.tensor.transpose(finT_ps, fin_T2, id_f32)
                res = consts.tile([NP, R + 1], F32)
                nc.vector.reduce_max(res[:, 0:1], finT_ps[:, R:RN],
                                     axis=mybir.AxisListType.X)
                nc.vector.tensor_copy(res[:, 1:R + 1], finT_ps[:, 0:R])
                nc.sync.dma_start(out=out[:, :], in_=res)

    return nc


def _host_shards(sequence_output, attention, relation_embeddings, nota_embeddings,
                 span_starts):
    rel_all = np.concatenate(
        [np.asarray(relation_embeddings, np.float32),
         np.asarray(nota_embeddings, np.float32)], axis=0)          # [77, 2304]
    relq = np.ascontiguousarray(
        rel_all.T.reshape(18, 128, RN).transpose(1, 0, 2).reshape(128, 18 * RN)
    ).astype(NP_BF16)

    in_maps = []
    for c in range(NCORES):
        b, g = divmod(c, 4)
        obj = list(range(8)) if g == 0 else list(range(8 * g, 8 * g + 8))
        ents = list(range(T)) + obj
        rows = np.concatenate(
            [np.arange(span_starts[b, e], span_starts[b, e] + W) for e in ents]
        )
        # attention rows -> [l, w, e, h] -> [128, (t w e h)]
        att_rows = attention[b][:, rows, :]                          # [H, 64, L]
        att_t = att_rows.reshape(H, NE, W, L).transpose(3, 2, 1, 0)  # [L, w, e, h]
        attq = (att_t.reshape(NT, 128, W * NE * H).transpose(1, 0, 2)
                .reshape(128, NT * W * NE * H)).astype(NP_BF16)
        # seq tiles + ones column -> [128, (t, 769)]
        st = sequence_output[b].reshape(NT, 128, D)
        st = np.concatenate([st, np.ones((NT, 128, 1), np.float32)], axis=2)
        seqq = st.transpose(1, 0, 2).reshape(128, NT * (D + 1)).astype(NP_BF16)
        # span rows transposed -> [d, row] chunks [128, (6, 64)]
        spT = sequence_output[b][rows].T.reshape(6, 128, NP).transpose(1, 0, 2)
        spansT = spT.reshape(128, 6 * NP).astype(NP_BF16)
        in_maps.append({
            "attq": np.ascontiguousarray(attq),
            "seqq": np.ascontiguousarray(seqq),
            "spansT": np.ascontiguousarray(spansT),
            "relq": relq,
        })
    return in_maps


def kernel(sequence_output, attention, relation_embeddings, nota_embeddings,
           span_starts):
    global LAST_RESULTS
    sequence_output = np.asarray(sequence_output, np.float32)
    attention = np.asarray(attention, np.float32)
    span_starts = np.asarray(span_starts)

    in_maps = _host_shards(sequence_output, attention, relation_embeddings,
                           nota_embeddings, span_starts)

    nc = _build_program()
    nc.finalize()
    LAST_RESULTS = run_bass_kernel_spmd(nc, in_maps, core_ids=list(range(NCORES)))

    out = np.zeros((B, len(ALL_PAIRS), R + 1), np.float32)
    for c in range(NCORES):
        b, g = divmod(c, 4)
        res = LAST_RESULTS.results[c]["out"]          # [64, 78], p = s*8+o_local
        idxs = GROUP_IDX[g]
        rows = [s * 8 + (o % 8) for (s, o) in (ALL_PAIRS[i] for i in idxs)]
        out[b, idxs, :] = res[rows]
    return out
